# revision 2
# baseline (speedup 1.0000x reference)
"""Trainium2 Bass kernel for CapsNet conv + dynamic-routing block.

Math note: in the reference, `pred` has a singleton MI axis, so the
softmax-weighted sum over MI is `pred` itself for any routing logits
(softmax rows sum to 1), and the `b` updates never change `c`.  The whole
module therefore reduces exactly to

    out = squash(conv2d_3x3(x2, conv_w) + conv_b)   # squash over DO

with x2 = x reshaped [B, MI*DI, H, W] and output [B, MO, H, W, DO].

Strategy: data-parallel over batch (1 image per NeuronCore, 8 cores).
Per core, the conv runs as 9 accumulating bf16 matmuls per 512-pixel
chunk ([ci,co] stationary, shifted xpad window moving), keeping the PE
stream pure matmul.  Everything else is off the PE:
  - bias add + f32->bf16 cast on ACT (PSUM -> SBUF),
  - [co,pix] -> [pix,co] transpose via the DMA crossbar (bf16),
  - squash: square on Pool, grouped reduce + factor on DVE (+ACT sqrt),
    final multiply alternating DVE/Pool,
  - DMA in/out split across the SP and ACT HWDGE rings.
"""

from contextlib import ExitStack

import numpy as np

import concourse.bass as bass
import concourse.mybir as mybir
import concourse.tile as tile
from concourse import bacc
from concourse.bass_utils import run_bass_kernel_spmd

B, MI, H, W, DI = 8, 8, 64, 64, 16
MO, DO = 8, 16
CI = MI * DI  # 128
CO = MO * DO  # 128
P = 128
HP, WP = H + 2, W + 2  # 66 (zero pad = 1)
NCHUNK = 8  # 512-pixel chunks per 64x64 image
EPS = 1e-7

F32 = mybir.dt.float32
BF16 = mybir.dt.bfloat16


def _body(tc, x_in, w_in, b_in, out_d, reps=1):
    import os

    variant = os.environ.get("KVAR", "full")
    nc = tc.nc
    with ExitStack() as ctx:
        consts = ctx.enter_context(tc.tile_pool(name="consts", bufs=1))
        cpsum = ctx.enter_context(tc.tile_pool(name="cpsum", bufs=6, space="PSUM"))
        spool = ctx.enter_context(tc.tile_pool(name="spool", bufs=4))
        sopool = ctx.enter_context(tc.tile_pool(name="sopool", bufs=4))
        sqpool = ctx.enter_context(tc.tile_pool(name="sqpool", bufs=4))
        facpool = ctx.enter_context(tc.tile_pool(name="facpool", bufs=3))
        outp = ctx.enter_context(tc.tile_pool(name="outp", bufs=4))

        zeros_sb = consts.tile([P, HP], BF16)
        nc.vector.memset(zeros_sb[:], 0.0)

        # weights: [ci, s, co] bf16 in SBUF (ACT ring, parallel with x on SP)
        w_sb = consts.tile([P, 9, CO], BF16)
        nc.scalar.dma_start(w_sb[:], w_in.rearrange("s ci co -> ci s co"))

        bias_sb = consts.tile([P, 1], F32)
        nc.scalar.dma_start(bias_sb[:], b_in)

        eps_sb = consts.tile([P, 1], F32)
        nc.vector.memset(eps_sb[:], EPS)

        # two padded images (bf16) so DMA-in of the next image overlaps the
        # current image's conv.  Borders zeroed once; loads touch interior.
        xpads = []
        for name in ("xpa", "xpb"):
            xp = consts.tile([P, HP, WP], BF16, tag=name)
            nc.scalar.copy(xp[:, 0, :], zeros_sb[:])
            nc.scalar.copy(xp[:, HP - 1, :], zeros_sb[:])
            nc.scalar.copy(xp[:, :, 0], zeros_sb[:])
            nc.scalar.copy(xp[:, :, WP - 1], zeros_sb[:])
            xpads.append(xp)

        def load_quarter(xp, g):
            """DMA 16 h-rows of x (contiguous source) into xpad rows 16g+1..16g+17."""
            nc.scalar.dma_start(
                xp[:, 1 + 16 * g : 17 + 16 * g, 1:65],
                x_in[:, 1024 * g : 1024 * g + 1024].rearrange(
                    "ci (r w) -> ci r w", w=W
                ),
            )

        def conv_pair(xp, c0):
            # conv for chunks c0, c0+1: 9 accumulating bf16 matmuls each,
            # s-outer so each weight load is reused back-to-back.
            ps0 = cpsum.tile([P, 4 * P], F32, tag="ps")
            ps1 = cpsum.tile([P, 4 * P], F32, tag="ps")
            nmm = 9 if variant != "dmaonly" else 1
            for s in range(nmm):
                kh, kw = s // 3, s % 3
                for ps, c in ((ps0, c0), (ps1, c0 + 1)):
                    rhs = xp[:, 8 * c + kh : 8 * c + kh + 8, kw : kw + 64]
                    nc.tensor.matmul(
                        ps[:],
                        w_sb[:, s, :],
                        rhs,
                        start=(s == 0),
                        stop=(s == nmm - 1),
                    )
            return ps0, ps1

        def post(c, ps, red):
            # bias add + cast to bf16 on ACT (PSUM -> SBUF)
            s_sb = spool.tile([P, 4 * P], BF16, tag="s_sb")
            nc.scalar.add(s_sb[:], ps[:], bias_sb[:])

            if variant in ("convonly", "dmaonly"):
                o = outp.tile([P, 4, P], F32, tag="out")
                nc.vector.tensor_copy(
                    o[:], s_sb[:].rearrange("p (t co) -> p t co", co=CO)
                )
                if variant != "nodma":
                    eng = nc.sync if c % 2 == 0 else nc.scalar
                    eng.dma_start(out_d[:, c], o[:])
                return None

            # transpose [co, pix] -> [pix, 4, co] on the DMA crossbar (SP ring)
            so = sopool.tile([P, 4, P], BF16, tag="so")
            nc.sync.dma_start_transpose(so[:], s_sb[:])

            # square on Pool (bf16 out), grouped sum over DO on DVE (f32)
            sq = sqpool.tile([P, 4, P], BF16, tag="sq")
            nc.gpsimd.tensor_mul(sq[:], so[:], so[:])
            nc.vector.tensor_reduce(
                red[:, c % 2],
                sq[:].rearrange("p t (g do) -> p (t g) do", do=DO),
                axis=mybir.AxisListType.X,
                op=mybir.AluOpType.add,
            )
            return so

        def squash_pair(c0, red, so_0, so_1):
            # factor = red / ((1+red) * sqrt(red+eps)), batched for 2 chunks
            r = facpool.tile([P, 2, 4 * MO], F32, tag="r")
            nc.scalar.activation(
                r[:], red[:], mybir.ActivationFunctionType.Sqrt, bias=eps_sb[:]
            )
            d = facpool.tile([P, 2, 4 * MO], F32, tag="d")
            nc.vector.scalar_tensor_tensor(
                d[:], red[:], 1.0, r[:], mybir.AluOpType.add, mybir.AluOpType.mult
            )
            rcp = facpool.tile([P, 2, 4 * MO], F32, tag="rcp")
            nc.vector.reciprocal(rcp[:], d[:])
            fac = facpool.tile([P, 2, 4 * MO], F32, tag="fac")
            nc.vector.tensor_mul(fac[:], red[:], rcp[:])

            for i, so in ((0, so_0), (1, so_1)):
                c = c0 + i
                o = outp.tile([P, 4, P], F32, tag="out")
                eng = nc.vector if i == 0 else nc.gpsimd
                eng.tensor_mul(
                    o[:].rearrange("p t (g do) -> p (t g) do", do=DO),
                    so[:].rearrange("p t (g do) -> p (t g) do", do=DO),
                    fac[:, i, :, None].to_broadcast((P, 4 * MO, DO)),
                )
                if variant != "nodma":
                    deng = nc.sync if i == 0 else nc.scalar
                    deng.dma_start(out_d[:, c], o[:])

        def one_image(xp, prefetch):
            """prefetch: list of 4 callables (or None), one per pair."""
            for p_ in range(4):
                if prefetch[p_] is not None and variant != "nodma":
                    prefetch[p_]()
                ps0, ps1 = conv_pair(xp, 2 * p_)
                red = facpool.tile([P, 2, 4 * MO], F32, tag="red")
                so_0 = post(2 * p_, ps0, red)
                so_1 = post(2 * p_ + 1, ps1, red)
                if variant not in ("convonly", "dmaonly"):
                    squash_pair(2 * p_, red, so_0, so_1)

        xa, xb = xpads

        if reps == 1:
            if variant != "nodma":
                for g in range(2):
                    load_quarter(xa, g)
            one_image(
                xa,
                [
                    (lambda: load_quarter(xa, 2)),
                    (lambda: load_quarter(xa, 3)),
                    None,
                    None,
                ],
            )
        else:
            # two images per hardware-loop iteration; image B's quarters
            # prefetch during A's conv and vice versa (cross-iteration).
            def body():
                one_image(
                    xa,
                    [
                        (lambda: load_quarter(xa, 2)),
                        (lambda: load_quarter(xa, 3)),
                        (lambda: load_quarter(xb, 0)),
                        (lambda: load_quarter(xb, 1)),
                    ],
                )
                one_image(
                    xb,
                    [
                        (lambda: load_quarter(xb, 2)),
                        (lambda: load_quarter(xb, 3)),
                        (lambda: load_quarter(xa, 0)),
                        (lambda: load_quarter(xa, 1)),
                    ],
                )

            if variant != "nodma":
                for g in range(2):
                    load_quarter(xa, g)
            with tc.For_i(0, reps // 2, 1):
                body()


_NC_CACHE = {}


def _get_nc(reps=1):
    key = ("nc", reps)
    if key not in _NC_CACHE:
        nc = bacc.Bacc("TRN2", target_bir_lowering=False, debug=False, num_devices=8)
        x_in = nc.dram_tensor("x", [CI, H * W], BF16, kind="ExternalInput").ap()
        w_in = nc.dram_tensor("w", [9, CI, CO], BF16, kind="ExternalInput").ap()
        b_in = nc.dram_tensor("bias", [CO, 1], F32, kind="ExternalInput").ap()
        out_d = nc.dram_tensor("out", [P, NCHUNK, 4, CO], F32, kind="ExternalOutput").ap()
        with tile.TileContext(nc) as tc:
            _body(tc, x_in, w_in, b_in, out_d, reps=reps)
        nc.compile()
        _NC_CACHE[key] = nc
    return _NC_CACHE[key]


def run(x, conv_w, conv_b, trace=False, reps=1):
    import ml_dtypes

    nc = _get_nc(reps=reps)
    # shard/prep: channel-major x per image, [ci, pix] contiguous, bf16
    xt = np.ascontiguousarray(
        np.asarray(x, dtype=np.float32)
        .transpose(0, 1, 4, 2, 3)
        .reshape(B, CI, H * W)
        .astype(ml_dtypes.bfloat16)
    )
    w9 = np.ascontiguousarray(
        np.asarray(conv_w, dtype=np.float32)
        .reshape(CO, CI, 9)
        .transpose(2, 1, 0)
        .astype(ml_dtypes.bfloat16)
    )
    bias = np.ascontiguousarray(np.asarray(conv_b, dtype=np.float32).reshape(CO, 1))
    in_maps = [{"x": xt[b], "w": w9, "bias": bias} for b in range(B)]
    res = run_bass_kernel_spmd(nc, in_maps, list(range(B)), trace=trace)
    # gather/unshard: out_dev[p, c, t, mo, do] -> out[b, mo, h, w, do]
    # with h = 8c + 2t + p//64, w = p%64  (pixel = 512c + 128t + p)
    dev = np.stack([res.results[i]["out"] for i in range(B)], axis=0)
    dev = dev.reshape(B, 2, W, NCHUNK, 4, MO, DO)  # [b, hl, w, c, t, mo, do]
    out = np.ascontiguousarray(
        dev.transpose(0, 5, 3, 4, 1, 2, 6).reshape(B, MO, H, W, DO)
    )
    return out, res


def kernel(x, conv_w, conv_b, b_logits=None, **_ignored):
    # b_logits provably has no effect on the reference output (see module
    # docstring), so it is accepted and ignored.
    out, _ = run(x, conv_w, conv_b, trace=False)
    return out
